# revision 1
# baseline (speedup 1.0000x reference)
"""BertSelfAttention Trainium2 Bass kernel.

Full (unsharded) inputs in, full output out. Internally shards across 8
NeuronCores as (batch b, head-group g): core c handles batch c//2 and
heads [6*(c%2), 6*(c%2)+6) of the 12 heads.

Per-core program (Tile framework):
  A) load hs[b], W/bias slices, mask[b]; PE-transpose to put the
     contraction dim on partitions (hsT [d,q], WT [d,out], maskT [k,1]).
  B) QT/KT [128=2 heads, 2048] via matmul; V [k, dh] directly (bias via
     rank-1 ones-row matmul); ones column appended per head for row-sums.
  C) per head, per q-chunk of 1024: flash-style loop over 16 k-tiles:
     scoresT [k-tile, q-chunk] in PSUM -> one ACT instruction does
     exp(0.125*s + mask_k) (scale folds 1/sqrt(64), per-partition bias
     folds the additive attention mask) -> probsT bf16 -> PV accumulates
     ctxT [65, q-chunk] in PSUM (row 64 = softmax denominator).
     Tail: PE-transpose ctxT -> [q, 65], DVE reciprocal + scale, DMA out.
"""

import os
import sys

sys.path.insert(0, "/opt/trn_rl_repo")

import numpy as np

B, S, D = 4, 2048, 768
H, DH = 12, 64
NCORES = 8
HPC = 6          # heads per core
GSZ = HPC * DH   # 384 output dims per core
P = 128
ND = D // P      # 6 d-tiles
NT = S // P      # 16 k-tiles
QC = 1024        # q-chunk
MMN = 512        # matmul free dim per instruction (fp32 limit)
MMN2 = 1024      # free dim for bf16 scores/PV matmuls

_cache = {}


def _build(mm_dt_name: str, loop_n: int = 0):
    key = (mm_dt_name, loop_n)
    if key in _cache:
        return _cache[key]

    import concourse.bass as bass
    import concourse.bacc as bacc
    import concourse.mybir as mybir
    from concourse import tile
    from concourse.masks import make_identity

    f32 = mybir.dt.float32
    mm_dt = getattr(mybir.dt, mm_dt_name)
    AF = mybir.ActivationFunctionType

    nc = bacc.Bacc("TRN2", target_bir_lowering=False, debug=False,
                   num_devices=NCORES)

    hs_d = nc.dram_tensor("hs", [S, D], f32, kind="ExternalInput")
    w_d = {p: nc.dram_tensor(f"w{p}", [GSZ, D], f32, kind="ExternalInput")
           for p in "qkv"}
    bias_d = nc.dram_tensor("bias", [3, GSZ], f32, kind="ExternalInput")
    mask_d = nc.dram_tensor("mask", [NT, P], f32, kind="ExternalInput")
    out_d = nc.dram_tensor("out", [S, GSZ], f32, kind="ExternalOutput")

    with tile.TileContext(nc) as tc:
        with tc.tile_pool(name="const", bufs=1) as const_pool, \
             tc.tile_pool(name="persist", bufs=1) as pers:

            ident = const_pool.tile([P, P], f32)
            make_identity(nc, ident[:])
            ident_mm = const_pool.tile([P, P], mm_dt)
            make_identity(nc, ident_mm[:])

            # ---- persistent SBUF tensors ----
            hsT = pers.tile([P, ND, S], mm_dt, tag="hsT")       # [d%128, dtile, q]
            wT = {p: pers.tile([P, ND, GSZ], mm_dt, tag=f"wT{p}", name=f"wT{p}")
                  for p in "qkv"}
            maskT = pers.tile([P, NT], f32, tag="maskT")        # [k%128, ktile]
            biasT = pers.tile([P, 6], f32, tag="biasT")         # [dim%128, pair*2+proj(q,k)]
            qT = pers.tile([P, 3, S], mm_dt, tag="qT")          # [2*dh, pair, q]
            kT = pers.tile([P, 3, S], mm_dt, tag="kT")
            vsb = pers.tile([P, NT, HPC * (DH + 1)], mm_dt, tag="vsb")
            bvrow = pers.tile([1, GSZ], mm_dt, tag="bvrow")
            onesrow = pers.tile([1, P], mm_dt, tag="onesrow")

            nc.vector.memset(vsb[:], 1.0)     # ones columns; v dims overwritten
            nc.vector.memset(onesrow[:], 1.0)

            import contextlib
            loop_cm = (tc.For_i(0, loop_n, 1,
                                hint_engines=(mybir.EngineType.PE,
                                              mybir.EngineType.Activation,
                                              mybir.EngineType.DVE,
                                              mybir.EngineType.SP))
                       if loop_n else contextlib.nullcontext())
            with loop_cm:
                # ================= Phase A: loads + transposes =================
                # single psum pool ("saps", 2x2-bank slots) serves phase A
                # transposes, phase B V-projection AND phase C score tiles:
                # sharing slots avoids the cross-phase bank-reuse WAR wall
                # that serialized A/B against C's start.
                with tc.tile_pool(name="stage", bufs=6) as stage, \
                     tc.tile_pool(name="saps", bufs=2, space="PSUM") as saps, \
                     tc.tile_pool(name="ctxps", bufs=2, space="PSUM") as ctxps, \
                     tc.tile_pool(name="tpps", bufs=2, space="PSUM") as tpps, \
                     tc.tile_pool(name="probs", bufs=6) as probs_pool, \
                     tc.tile_pool(name="tailsb", bufs=2) as tailsb, \
                     tc.tile_pool(name="outsb", bufs=4) as outsb:

                    # mask [NT, P] -> maskT [P, NT]
                    mstage = stage.tile([NT, P], f32, tag="mstage")
                    nc.sync.dma_start(mstage[:], mask_d[:])
                    mps = saps.tile([P, NT], f32, tag="sa", name="mps")
                    nc.tensor.transpose(mps[:], mstage[:], ident[:NT, :NT])
                    nc.vector.tensor_copy(maskT[:], mps[:])

                    # bias [3, GSZ] -> biasT [P, proj(q,k), pair]; bv -> bvrow
                    bstage = stage.tile([3, GSZ], f32, tag="bstage")
                    nc.sync.dma_start(bstage[:], bias_d[:])
                    for pp in range(3):
                        bps = saps.tile([P, 3], f32, tag="sa", name="bps")
                        nc.tensor.transpose(bps[:], bstage[:, pp * P:(pp + 1) * P],
                                            ident[:3, :3])
                        nc.vector.tensor_copy(biasT[:, pp * 2:pp * 2 + 2], bps[:, 0:2])
                    bvstage = stage.tile([1, GSZ], f32, tag="bvstage")
                    nc.sync.dma_start(bvstage[:], bias_d[2:3, :])
                    nc.vector.tensor_copy(bvrow[0:1, :], bvstage[0:1, :])

                    # hs -> hsT (cast to mm_dt on one batched eviction per tile)
                    for t in range(NT):
                        hstage = stage.tile([P, D], f32, tag="hstage")
                        nc.sync.dma_start(hstage[:], hs_d[t * P:(t + 1) * P, :])
                        ps = saps.tile([P, D], f32, tag="sa", name="trb")
                        for d in range(ND):
                            nc.tensor.transpose(ps[:, d * P:(d + 1) * P],
                                                hstage[:, d * P:(d + 1) * P],
                                                ident[:])
                        nc.vector.tensor_copy(
                            hsT[:, :, t * P:(t + 1) * P],
                            ps[:].rearrange("p (d c) -> p d c", c=P))

                    # W slices -> wT
                    for p in "qkv":
                        for r in range(GSZ // P):  # 3 row-tiles of 128 outdims
                            wstage = stage.tile([P, D], f32, tag="wstage")
                            nc.sync.dma_start(wstage[:], w_d[p][r * P:(r + 1) * P, :])
                            ps = saps.tile([P, D], f32, tag="sa", name="trb")
                            for d in range(ND):
                                nc.tensor.transpose(ps[:, d * P:(d + 1) * P],
                                                    wstage[:, d * P:(d + 1) * P],
                                                    ident[:])
                            nc.vector.tensor_copy(
                                wT[p][:, :, r * P:(r + 1) * P],
                                ps[:].rearrange("p (d c) -> p d c", c=P))

                    # ================= Phase B: V projection =================
                    if True:
                        # V [k, dh] per k-tile (+ bias via rank-1 ones x bv)
                        for t in range(NT):
                            ps = saps.tile([P, GSZ], f32, tag="sa", name="vp")
                            for d in range(ND):
                                nc.tensor.matmul(ps[:], hsT[:, d, t * P:(t + 1) * P],
                                                 wT["v"][:, d, :],
                                                 start=(d == 0), stop=False)
                            nc.tensor.matmul(ps[:], onesrow[0:1, :],
                                             bvrow[0:1, :], start=False, stop=True)
                            nc.vector.tensor_copy(
                                vsb[:, t, :].rearrange("p (h c) -> p h c", c=DH + 1)[:, :, 0:DH],
                                ps[:].rearrange("p (h c) -> p h c", c=DH))

                    # ================= Phase C: attention per head =================
                    # QC2=512: scores psum tiles are 1 bank each -> 4 concurrent
                    # slots allow both heads of a pair to issue score matmuls
                    # adjacently (row-group packed on PE).
                    if True:
                        QC2 = 512
                        for pp3 in range(3):
                            # QT / KT for this head-pair (1-bank psum chunks)
                            for pi, (pname, dst) in enumerate((("q", qT), ("k", kT))):
                                for ch in range(S // QC2):
                                    qkp = saps.tile([P, QC2], f32, tag="sa", name="qkp")
                                    for d in range(ND):
                                        nc.tensor.matmul(
                                            qkp[:],
                                            wT[pname][:, d, pp3 * P:(pp3 + 1) * P],
                                            hsT[:, d, ch * QC2:(ch + 1) * QC2],
                                            start=(d == 0), stop=(d == ND - 1))
                                    nc.vector.tensor_scalar_add(
                                        dst[:, pp3, ch * QC2:(ch + 1) * QC2],
                                        qkp[:], biasT[:, pp3 * 2 + pi:pp3 * 2 + pi + 1])

                            hA, hB = 2 * pp3, 2 * pp3 + 1
                            for qc in range(S // QC2):
                                ctxs = {}
                                for h in (hA, hB):
                                    ctxs[h] = ctxps.tile([DH + 1, QC2], f32, tag="ctx",
                                                         name=f"ctx{h}")
                                for t in range(NT):
                                    # both heads' scores land in ONE [128, 1024]
                                    # psum tile (halves = different banks, written
                                    # by row-group-packed matmuls) so a single
                                    # ACT instruction exps both heads at once
                                    sa = saps.tile([P, 2 * QC2], f32, tag="sa",
                                                   name="sa")
                                    for i, h in enumerate((hA, hB)):
                                        base = (h % 2) * DH
                                        nc.tensor.matmul(
                                            sa[:, i * QC2:(i + 1) * QC2],
                                            kT[base:base + DH, pp3, t * P:(t + 1) * P],
                                            qT[base:base + DH, pp3,
                                               qc * QC2:(qc + 1) * QC2],
                                            start=True, stop=True)
                                    pr = probs_pool.tile([P, 2 * QC2], mm_dt,
                                                         tag="pr", name="pr")
                                    nc.scalar.activation(pr[:], sa[:], AF.Exp,
                                                         bias=maskT[:, t:t + 1],
                                                         scale=0.125)
                                    for i, h in enumerate((hA, hB)):
                                        nc.tensor.matmul(
                                            ctxs[h][:],
                                            vsb[:, t, h * (DH + 1):(h + 1) * (DH + 1)],
                                            pr[:, i * QC2:(i + 1) * QC2],
                                            start=(t == 0), stop=(t == NT - 1))
                                # tail: normalize + transpose + store
                                for h in (hA, hB):
                                    ctxu = tailsb.tile([DH + 1, QC2], f32, tag="ctxu",
                                                       name=f"ctxu{h}")
                                    nc.vector.tensor_copy(ctxu[:], ctxs[h][:])
                                    for s2 in range(QC2 // P):
                                        tp = tpps.tile([P, DH + 1], f32, tag="tp")
                                        nc.tensor.transpose(
                                            tp[:], ctxu[:, s2 * P:(s2 + 1) * P],
                                            ident[:DH + 1, :DH + 1])
                                        rcp = outsb.tile([P, 1], f32, tag="rcp")
                                        nc.vector.reciprocal(rcp[:], tp[:, DH:DH + 1])
                                        ot = outsb.tile([P, DH], f32, tag="ot")
                                        nc.vector.tensor_scalar_mul(ot[:], tp[:, 0:DH],
                                                                    rcp[:])
                                        q0 = qc * QC2 + s2 * P
                                        nc.sync.dma_start(
                                            out_d[q0:q0 + P, h * DH:(h + 1) * DH],
                                            ot[:])

    nc.compile()
    _cache[key] = nc
    return nc


def _in_maps(hidden_states, attention_mask, Wq, bq, Wk, bk, Wv, bv):
    maps = []
    for c in range(NCORES):
        b, g = c // 2, c % 2
        sl = slice(g * GSZ, (g + 1) * GSZ)
        maps.append({
            "hs": np.ascontiguousarray(hidden_states[b], dtype=np.float32),
            "wq": np.ascontiguousarray(Wq[sl], dtype=np.float32),
            "wk": np.ascontiguousarray(Wk[sl], dtype=np.float32),
            "wv": np.ascontiguousarray(Wv[sl], dtype=np.float32),
            "bias": np.ascontiguousarray(
                np.stack([bq[sl], bk[sl], bv[sl]]), dtype=np.float32),
            "mask": np.ascontiguousarray(
                attention_mask[b].reshape(NT, P), dtype=np.float32),
        })
    return maps


def kernel(hidden_states, attention_mask, Wq, bq, Wk, bk, Wv, bv,
           _trace=False, _tmpdir=None):
    from concourse.bass_utils import run_bass_kernel_spmd

    nc = _build(os.environ.get("BERT_MM_DT", "bfloat16"))
    maps = _in_maps(np.asarray(hidden_states), np.asarray(attention_mask),
                    np.asarray(Wq), np.asarray(bq), np.asarray(Wk),
                    np.asarray(bk), np.asarray(Wv), np.asarray(bv))
    res = run_bass_kernel_spmd(nc, maps, core_ids=list(range(NCORES)),
                               trace=_trace, tmpdir=_tmpdir)
    out = np.empty((B, S, D), dtype=np.float32)
    for c in range(NCORES):
        b, g = c // 2, c % 2
        out[b, :, g * GSZ:(g + 1) * GSZ] = res.results[c]["out"]
    kernel.last_results = res
    return out



# revision 22
# speedup vs baseline: 1.2120x; 1.2120x over previous
"""BertSelfAttention Trainium2 Bass kernel.

Full (unsharded) inputs in, full output out. Internally shards across 8
NeuronCores as (batch b, head-group g): core c handles batch c//2 and
heads [6*(c%2), 6*(c%2)+6) of the 12 heads.

Per-core program (Tile framework), designed to keep the Activation
engine (the exp bottleneck: 192 x [128,1024] exp instructions ~= 199us)
saturated:

  A) DMA hs/W/bias/mask; cast to bf16 on DVE, PE-transpose (1 cyc/row)
     into hsT [d,q] / wT [d,out]; mask/bias transposed in f32.
  B) kT/qT per head-pair via matmul chains (d-contraction in a shared
     score-psum slot), V [k, 65*(h)] chains (ones column appended per
     head for the softmax denominator, bias via rank-1 ones x bv).
  C) attention per pair, q-chunks of 512: per k-tile t, both heads'
     scoresT [k,512|512] land in one [128,1024] psum tile; one ACT
     instruction computes exp(0.125*s + mask_k) -> pr bf16 in SBUF.
     PV is output-stationary: ctx[q,65] accumulates pr^T (stationary)
     x [V_h | 1] over t — 65-row matmuls, no tail transpose. Tail:
     DVE reciprocal of the ones-column + scale, pair-batched DMA out.
  All phase-A/B prep that is not needed for the first scores is queued
  and drained inside the attention loop so the PE prep work hides
  behind ACT exp time instead of serializing in front of it.
"""

import os
import sys

sys.path.insert(0, "/opt/trn_rl_repo")

import numpy as np

B, S, D = 4, 2048, 768
H, DH = 12, 64
NCORES = 8
HPC = 6          # heads per core
GSZ = HPC * DH   # 384 output dims per core
P = 128
ND = D // P      # 6 d-tiles
NT = S // P      # 16 k-tiles
QC2 = 512        # q-chunk (scores matmul free dim; fp32-psum limit)
NQC = S // QC2   # 4
DH1 = DH + 1     # 65: v dims + ones column

_cache = {}


def _build(mm_dt_name: str, loop_n: int = 0):
    key = (mm_dt_name, loop_n)
    if key in _cache:
        return _cache[key]

    import concourse.bass as bass
    import concourse.bacc as bacc
    import concourse.mybir as mybir
    from concourse import tile
    from concourse.masks import make_identity

    f32 = mybir.dt.float32
    mm_dt = getattr(mybir.dt, mm_dt_name)
    AF = mybir.ActivationFunctionType

    nc = bacc.Bacc("TRN2", target_bir_lowering=False, debug=False,
                   num_devices=NCORES)

    hs_d = nc.dram_tensor("hs", [S, D], f32, kind="ExternalInput")
    w_d = {p: nc.dram_tensor(f"w{p}", [GSZ, D], f32, kind="ExternalInput")
           for p in "qkv"}
    bias_d = nc.dram_tensor("bias", [3, GSZ], f32, kind="ExternalInput")
    mask_d = nc.dram_tensor("mask", [NT, P], f32, kind="ExternalInput")
    out_d = nc.dram_tensor("out", [S, GSZ], f32, kind="ExternalOutput")
    dbg_d = (nc.dram_tensor("dbg", [QC2, 2 * DH1], f32, kind="ExternalOutput")
             if os.environ.get("BERT_DBG") else None)

    with tile.TileContext(nc) as tc:
        with tc.tile_pool(name="const", bufs=1) as const_pool, \
             tc.tile_pool(name="persist", bufs=1) as pers:

            ident = const_pool.tile([P, P], f32)
            make_identity(nc, ident[:])
            ident_mm = const_pool.tile([P, P], mm_dt)
            make_identity(nc, ident_mm[:])

            # ---- persistent SBUF tensors ----
            hsT = pers.tile([P, ND, S], mm_dt, tag="hsT")       # [d%128, dtile, q]
            wT = {p: pers.tile([P, ND, GSZ], mm_dt, tag=f"wT{p}", name=f"wT{p}")
                  for p in "qkv"}
            maskT = pers.tile([P, NT], f32, tag="maskT")        # [k%128, ktile]
            biasT = pers.tile([P, 6], f32, tag="biasT")         # [dim%128, pair*2+proj(q,k)]
            qT = pers.tile([P, 3, S], mm_dt, tag="qT")          # [2*dh, pair, q]
            kT = pers.tile([P, 3, S], mm_dt, tag="kT")
            vsb = pers.tile([P, NT, HPC * DH1], mm_dt, tag="vsb")
            bvrow = pers.tile([1, GSZ], mm_dt, tag="bvrow")
            onesrow = pers.tile([1, P], mm_dt, tag="onesrow")

            nc.vector.memset(vsb[:], 1.0)     # ones columns; v dims overwritten
            nc.vector.memset(onesrow[:], 1.0)

            import contextlib
            loop_cm = (tc.For_i(0, loop_n, 1,
                                hint_engines=(mybir.EngineType.PE,
                                              mybir.EngineType.Activation,
                                              mybir.EngineType.DVE,
                                              mybir.EngineType.SP))
                       if loop_n else contextlib.nullcontext())
            with loop_cm:
                with tc.tile_pool(name="hstage", bufs=16) as hstage_pool, \
                     tc.tile_pool(name="wstage", bufs=9) as wstage_pool, \
                     tc.tile_pool(name="cast", bufs=3) as cast_pool, \
                     tc.tile_pool(name="stage2", bufs=1) as stage2, \
                     tc.tile_pool(name="sa", bufs=2, space="PSUM") as sa_pool, \
                     tc.tile_pool(name="scr", bufs=1, space="PSUM") as scr_pool, \
                     tc.tile_pool(name="ctxp", bufs=3, space="PSUM") as ctx_pool, \
                     tc.tile_pool(name="prp", bufs=6) as pr_pool, \
                     tc.tile_pool(name="outsb", bufs=4) as out_pool:

                    # ---------- DMA issue (order = need order) ----------
                    mstage = stage2.tile([NT, P], f32, tag="mstage")
                    nc.sync.dma_start(mstage[:], mask_d[:])
                    bstage = stage2.tile([3, GSZ], f32, tag="bstage")
                    nc.sync.dma_start(bstage[:], bias_d[:])

                    hstages = {}
                    wstages = {}

                    def dma_h(j):
                        t_ = hstage_pool.tile([P, D], f32, tag="hstage",
                                              name=f"hst{j}")
                        nc.sync.dma_start(t_[:], hs_d[j * P:(j + 1) * P, :])
                        hstages[j] = t_

                    def dma_w(p, r):
                        t_ = wstage_pool.tile([P, D], f32, tag="wstage",
                                              name=f"wst{p}{r}")
                        nc.sync.dma_start(t_[:], w_d[p][r * P:(r + 1) * P, :])
                        wstages[(p, r)] = t_

                    dma_w("q", 0); dma_w("k", 0)
                    for j in range(4):
                        dma_h(j)
                    for r in range(3):
                        dma_w("v", r)
                    for j in range(4, 8):
                        dma_h(j)
                    dma_w("q", 1); dma_w("k", 1)
                    for j in range(8, 12):
                        dma_h(j)
                    dma_w("q", 2); dma_w("k", 2)
                    for j in range(12, 16):
                        dma_h(j)

                    # mask [NT, P] -> maskT [P, NT]
                    mps = scr_pool.tile([P, NT], f32, tag="scr", name="mps")
                    nc.tensor.transpose(mps[:], mstage[:], ident[:NT, :NT])
                    nc.vector.tensor_copy(maskT[:], mps[:])

                    # bias [3, GSZ] -> biasT [P, pair*2+proj]; bv -> bvrow
                    for pp in range(3):
                        bps = scr_pool.tile([P, 3], f32, tag="scr", name="bps")
                        nc.tensor.transpose(bps[:], bstage[:, pp * P:(pp + 1) * P],
                                            ident[:3, :3])
                        nc.vector.tensor_copy(biasT[:, pp * 2:pp * 2 + 2], bps[:, 0:2])
                    bvstage = stage2.tile([1, GSZ], f32, tag="bvstage")
                    nc.sync.dma_start(bvstage[:], bias_d[2:3, :])
                    nc.vector.tensor_copy(bvrow[0:1, :], bvstage[0:1, :])

                    # ---------- transpose / projection helpers ----------
                    def tp_hs(j):
                        hc = cast_pool.tile([P, D], mm_dt, tag="hcast",
                                            name=f"hc{j}")
                        nc.vector.tensor_copy(hc[:], hstages[j][:])
                        ps = scr_pool.tile([P, D], mm_dt, tag="scr", name=f"tph{j}")
                        for d in range(ND):
                            nc.tensor.transpose(ps[:, d * P:(d + 1) * P],
                                                hc[:, d * P:(d + 1) * P],
                                                ident_mm[:])
                        nc.vector.tensor_copy(
                            hsT[:, :, j * P:(j + 1) * P],
                            ps[:].rearrange("p (d c) -> p d c", c=P))

                    def tp_w(p, r):
                        wc = cast_pool.tile([P, D], mm_dt, tag="hcast",
                                            name=f"wc{p}{r}")
                        nc.vector.tensor_copy(wc[:], wstages[(p, r)][:])
                        ps = scr_pool.tile([P, D], mm_dt, tag="scr",
                                           name=f"tpw{p}{r}")
                        for d in range(ND):
                            nc.tensor.transpose(ps[:, d * P:(d + 1) * P],
                                                wc[:, d * P:(d + 1) * P],
                                                ident_mm[:])
                        nc.vector.tensor_copy(
                            wT[p][:, :, r * P:(r + 1) * P],
                            ps[:].rearrange("p (d c) -> p d c", c=P))

                    # kT/qT chunk projection on the scratch bank: the score
                    # psum slots stay exclusive to scores so a projection
                    # chain never delays the exp pipeline.
                    def proj_qk(pname, dst, pp3, ch, pi):
                        ps = scr_pool.tile([P, QC2], f32, tag="scr",
                                           name=f"pj{pname}{pp3}{ch}")
                        for d in range(ND):
                            nc.tensor.matmul(ps[:],
                                             wT[pname][:, d, pp3 * P:(pp3 + 1) * P],
                                             hsT[:, d, ch * QC2:(ch + 1) * QC2],
                                             start=(d == 0), stop=(d == ND - 1))
                        nc.vector.tensor_scalar_add(
                            dst[:, pp3, ch * QC2:(ch + 1) * QC2],
                            ps[:], biasT[:, pp3 * 2 + pi:pp3 * 2 + pi + 1])

                    def proj_v(t):
                        ps = scr_pool.tile([P, GSZ], f32, tag="scr",
                                           name=f"pv{t}")
                        for d in range(ND):
                            nc.tensor.matmul(ps[:],
                                             hsT[:, d, t * P:(t + 1) * P],
                                             wT["v"][:, d, :],
                                             start=(d == 0), stop=False)
                        nc.tensor.matmul(ps[:], onesrow[0:1, :],
                                         bvrow[0:1, :], start=False, stop=True)
                        nc.vector.tensor_copy(
                            vsb[:, t, :].rearrange("p (h c) -> p h c", c=DH1)[:, :, 0:DH],
                            ps[:].rearrange("p (h c) -> p h c", c=DH))

                    # ---------- A0: critical prep for the first scores ----------
                    for j in range(4):
                        tp_hs(j)
                    tp_w("q", 0); tp_w("k", 0)
                    proj_qk("k", kT, 0, 0, 1)
                    proj_qk("q", qT, 0, 0, 0)
                    for r in range(3):
                        tp_w("v", r)
                    proj_v(0)

                    # ---------- deferred prep queue ----------
                    # Items carry an absolute "need-by" period (pair*64 +
                    # qc*16 + t): the item is force-issued by the END of
                    # period need-1 so the first consumer (period `need`)
                    # sees an earlier-issued producer. Uniform spreading
                    # drains ahead of the deadlines when PE has slack.
                    prep = []
                    def q_(need, fn, *a):
                        prep.append((need, fn, a))
                    # pair-0 qc0: remaining hsT tiles, kT chunks, V tiles
                    q_(3, tp_hs, 4); q_(3, tp_hs, 5); q_(1, proj_v, 1)
                    q_(3, tp_hs, 6); q_(2, proj_v, 2); q_(3, tp_hs, 7)
                    q_(3, proj_v, 3); q_(4, proj_qk, "k", kT, 0, 1, 1)
                    q_(4, proj_v, 4); q_(7, tp_hs, 8); q_(5, proj_v, 5)
                    q_(7, tp_hs, 9); q_(6, proj_v, 6); q_(7, tp_hs, 10)
                    q_(7, proj_v, 7); q_(7, tp_hs, 11)
                    q_(8, proj_qk, "k", kT, 0, 2, 1); q_(8, proj_v, 8)
                    q_(11, tp_hs, 12); q_(9, proj_v, 9); q_(11, tp_hs, 13)
                    q_(10, proj_v, 10); q_(11, tp_hs, 14); q_(11, proj_v, 11)
                    q_(11, tp_hs, 15); q_(12, proj_qk, "k", kT, 0, 3, 1)
                    q_(12, proj_v, 12); q_(13, proj_v, 13); q_(14, proj_v, 14)
                    q_(15, proj_v, 15); q_(16, proj_qk, "q", qT, 0, 1, 0)
                    n_qc0 = len(prep)
                    # pair-0 qc1: qT c2 + W transposes for pair 1
                    q_(32, proj_qk, "q", qT, 0, 2, 0)
                    q_(46, tp_w, "q", 1); q_(46, tp_w, "k", 1)
                    n_qc1 = len(prep)
                    # pair-0 qc2: qT c3 + pair-1 kT
                    q_(48, proj_qk, "q", qT, 0, 3, 0)
                    for c in range(4):
                        q_(64 + 4 * c, proj_qk, "k", kT, 1, c, 1)
                    n_qc2 = len(prep)
                    # pair-0 qc3: pair-1 qT
                    for c in range(4):
                        q_(64 + 16 * c if c else 64, proj_qk, "q", qT, 1, c, 0)
                    n_qc3 = len(prep)
                    # pair-1 qc0: W transposes for pair 2
                    q_(100, tp_w, "q", 2); q_(100, tp_w, "k", 2)
                    n_qc4 = len(prep)
                    # pair-1 qc1: pair-2 kT
                    for c in range(4):
                        q_(128 + 4 * c, proj_qk, "k", kT, 2, c, 1)
                    n_qc5 = len(prep)
                    # pair-1 qc2: pair-2 qT
                    for c in range(4):
                        q_(128 + 16 * c if c else 128, proj_qk, "q", qT, 2, c, 0)
                    n_qc6 = len(prep)

                    # cumulative prep targets per (pair, qc)
                    targets = {(0, 0): n_qc0, (0, 1): n_qc1, (0, 2): n_qc2,
                               (0, 3): n_qc3, (1, 0): n_qc4, (1, 1): n_qc5,
                               (1, 2): n_qc6}
                    # suffix-min effective needs: an early item can never
                    # head-of-line-block a later item with a tighter deadline
                    # (issuing earlier is always safe; order is preserved)
                    eff = [0] * len(prep)
                    mn = 1 << 30
                    for idx in range(len(prep) - 1, -1, -1):
                        mn = min(mn, prep[idx][0])
                        eff[idx] = mn
                    prep = [(eff[idx], fn, a)
                            for idx, (need, fn, a) in enumerate(prep)]
                    state = {"done": 0}
                    mode = os.environ.get("BERT_SERIAL_PREP", "")
                    if mode:
                        # debug bisect: run selected prep kinds serially now,
                        # keep the rest interleaved
                        keep = []
                        for need, fn, a in prep:
                            kind = {tp_hs: "h", tp_w: "w", proj_v: "v",
                                    proj_qk: "k"}[fn]
                            if mode == "all" or kind in mode:
                                fn(*a)
                            else:
                                keep.append((need, fn, a))
                        prep[:] = keep

                    def drain_prep(p_abs, hi):
                        while state["done"] < len(prep):
                            need, fn, a = prep[state["done"]]
                            if state["done"] >= hi and need > p_abs + 1:
                                break
                            fn(*a)
                            state["done"] += 1

                    # ---------- Phase C: attention ----------
                    for pp3 in range(3):
                        for qc in range(NQC):
                            base_done = state["done"]
                            tgt = targets.get((pp3, qc), state["done"])
                            # 4 qtile accumulation regions share each ctx bank;
                            # psum start-bits zero a whole 2KB bank, so zero
                            # via DVE and accumulate with start=False instead.
                            ctxs = {}
                            for i in (0, 1):
                                ctxs[i] = ctx_pool.tile([P, QC2], f32, tag="ctx",
                                                        name=f"ctx{pp3}{qc}{i}")
                                nc.vector.memset(ctxs[i][:], 0.0)
                            for t in range(NT):
                                sa = sa_pool.tile([P, 2 * QC2], f32, tag="sa",
                                                  name="sa")
                                for i in (0, 1):
                                    base = i * DH
                                    nc.tensor.matmul(
                                        sa[:, i * QC2:(i + 1) * QC2],
                                        kT[base:base + DH, pp3, t * P:(t + 1) * P],
                                        qT[base:base + DH, pp3,
                                           qc * QC2:(qc + 1) * QC2],
                                        start=True, stop=True)
                                pr = pr_pool.tile([P, 2 * QC2], mm_dt,
                                                  tag="pr", name="pr")
                                nc.scalar.activation(pr[:], sa[:], AF.Exp,
                                                     bias=maskT[:, t:t + 1],
                                                     scale=0.125)
                                for i in (0, 1):
                                    h = 2 * pp3 + i
                                    for j in range(QC2 // P):
                                        nc.tensor.matmul(
                                            ctxs[i][:, j * DH1:(j + 1) * DH1],
                                            pr[:, i * QC2 + j * P:
                                               i * QC2 + (j + 1) * P],
                                            vsb[:, t, h * DH1:(h + 1) * DH1],
                                            start=False, stop=(t == NT - 1),
                                            skip_group_check=True)
                                # spread deferred prep across the t-loop
                                p_abs = pp3 * 64 + qc * NT + t
                                drain_prep(p_abs,
                                           base_done + ((tgt - base_done)
                                                        * (t + 1) + NT - 1) // NT)
                            # tail: normalize + pair-batched store
                            if dbg_d is not None and pp3 == 0 and qc == 0:
                                for j in range(QC2 // P):
                                    for i in (0, 1):
                                        dstage = out_pool.tile([P, DH1], f32,
                                                               tag="dbg", name="dbg")
                                        nc.vector.tensor_copy(
                                            dstage[:],
                                            ctxs[i][:, j * DH1:(j + 1) * DH1])
                                        nc.sync.dma_start(
                                            dbg_d[j * P:(j + 1) * P,
                                                  i * DH1:(i + 1) * DH1],
                                            dstage[:])
                            for j in range(QC2 // P):
                                ot = out_pool.tile([P, P], f32, tag="ot",
                                                   name="ot")
                                for i in (0, 1):
                                    rcp = out_pool.tile([P, 1], f32, tag="rcp",
                                                        name="rcp")
                                    nc.vector.reciprocal(
                                        rcp[:],
                                        ctxs[i][:, j * DH1 + DH:(j + 1) * DH1])
                                    nc.vector.tensor_scalar_mul(
                                        ot[:, i * DH:(i + 1) * DH],
                                        ctxs[i][:, j * DH1:j * DH1 + DH],
                                        rcp[:])
                                q0 = qc * QC2 + j * P
                                nc.sync.dma_start(
                                    out_d[q0:q0 + P, pp3 * P:(pp3 + 1) * P],
                                    ot[:])

    nc.compile()
    _cache[key] = nc
    return nc


def _in_maps(hidden_states, attention_mask, Wq, bq, Wk, bk, Wv, bv):
    maps = []
    for c in range(NCORES):
        b, g = c // 2, c % 2
        sl = slice(g * GSZ, (g + 1) * GSZ)
        maps.append({
            "hs": np.ascontiguousarray(hidden_states[b], dtype=np.float32),
            "wq": np.ascontiguousarray(Wq[sl], dtype=np.float32),
            "wk": np.ascontiguousarray(Wk[sl], dtype=np.float32),
            "wv": np.ascontiguousarray(Wv[sl], dtype=np.float32),
            "bias": np.ascontiguousarray(
                np.stack([bq[sl], bk[sl], bv[sl]]), dtype=np.float32),
            "mask": np.ascontiguousarray(
                attention_mask[b].reshape(NT, P), dtype=np.float32),
        })
    return maps


def kernel(hidden_states, attention_mask, Wq, bq, Wk, bk, Wv, bv,
           _trace=False, _tmpdir=None):
    from concourse.bass_utils import run_bass_kernel_spmd

    nc = _build(os.environ.get("BERT_MM_DT", "bfloat16"))
    maps = _in_maps(np.asarray(hidden_states), np.asarray(attention_mask),
                    np.asarray(Wq), np.asarray(bq), np.asarray(Wk),
                    np.asarray(bk), np.asarray(Wv), np.asarray(bv))
    res = run_bass_kernel_spmd(nc, maps, core_ids=list(range(NCORES)),
                               trace=_trace, tmpdir=_tmpdir)
    out = np.empty((B, S, D), dtype=np.float32)
    for c in range(NCORES):
        b, g = c // 2, c % 2
        out[b, :, g * GSZ:(g + 1) * GSZ] = res.results[c]["out"]
    kernel.last_results = res
    return out


# revision 35
# speedup vs baseline: 1.3073x; 1.0786x over previous
"""BertSelfAttention Trainium2 Bass kernel.

Full (unsharded) inputs in, full output out. Internally shards across 8
NeuronCores as (batch b, head-group g): core c handles batch c//2 and
heads [6*(c%2), 6*(c%2)+6) of the 12 heads.

Per-core program (Tile framework), designed to keep the Activation
engine (the exp bottleneck: 192 x [128,1024] exp instructions ~= 199us)
saturated:

  A) DMA hs/W/bias/mask; cast to bf16 on DVE, PE-transpose (1 cyc/row)
     into hsT [d,q] / wT [d,out]; mask/bias transposed in f32.
  B) kT/qT per head-pair via matmul chains (d-contraction in a shared
     score-psum slot), V [k, 65*(h)] chains (ones column appended per
     head for the softmax denominator, bias via rank-1 ones x bv).
  C) attention per pair, q-chunks of 512: per k-tile t, both heads'
     scoresT [k,512|512] land in one [128,1024] psum tile; one ACT
     instruction computes exp(0.125*s + mask_k) -> pr bf16 in SBUF.
     PV is output-stationary: ctx[q,65] accumulates pr^T (stationary)
     x [V_h | 1] over t — 65-row matmuls, no tail transpose. Tail:
     DVE reciprocal of the ones-column + scale, pair-batched DMA out.
  All phase-A/B prep that is not needed for the first scores is queued
  and drained inside the attention loop so the PE prep work hides
  behind ACT exp time instead of serializing in front of it.
"""

import os
import sys

sys.path.insert(0, "/opt/trn_rl_repo")

import numpy as np

B, S, D = 4, 2048, 768
H, DH = 12, 64
NCORES = 8
HPC = 6          # heads per core
GSZ = HPC * DH   # 384 output dims per core
P = 128
ND = D // P      # 6 d-tiles
NT = S // P      # 16 k-tiles
QC2 = 512        # q-chunk (scores matmul free dim; fp32-psum limit)
NQC = S // QC2   # 4
DH1 = DH + 1     # 65: v dims + ones column

_cache = {}


def _build(mm_dt_name: str, loop_n: int = 0):
    key = (mm_dt_name, loop_n)
    if key in _cache:
        return _cache[key]

    import concourse.bass as bass
    import concourse.bacc as bacc
    import concourse.mybir as mybir
    from concourse import tile
    from concourse.masks import make_identity

    f32 = mybir.dt.float32
    mm_dt = getattr(mybir.dt, mm_dt_name)
    AF = mybir.ActivationFunctionType

    nc = bacc.Bacc("TRN2", target_bir_lowering=False, debug=False,
                   num_devices=NCORES)

    # hs/W arrive pre-cast to the matmul dtype (host-side prep) so the DMA
    # XBAR can transpose straight out of DRAM (2-byte dtype requirement).
    hs_d = nc.dram_tensor("hs", [S, D], mm_dt, kind="ExternalInput")
    w_d = {p: nc.dram_tensor(f"w{p}", [GSZ, D], mm_dt, kind="ExternalInput")
           for p in "qkv"}
    bias_d = nc.dram_tensor("bias", [3, GSZ], f32, kind="ExternalInput")
    mask_d = nc.dram_tensor("mask", [NT, P], f32, kind="ExternalInput")
    out_d = nc.dram_tensor("out", [S, GSZ], f32, kind="ExternalOutput")
    dbg_d = (nc.dram_tensor("dbg", [QC2, 2 * DH1], f32, kind="ExternalOutput")
             if os.environ.get("BERT_DBG") else None)

    with tile.TileContext(nc) as tc:
        with tc.tile_pool(name="const", bufs=1) as const_pool, \
             tc.tile_pool(name="persist", bufs=1) as pers:

            ident = const_pool.tile([P, P], f32)
            make_identity(nc, ident[:])
            ident_mm = const_pool.tile([P, P], mm_dt)
            make_identity(nc, ident_mm[:])

            # ---- persistent SBUF tensors ----
            hsT = pers.tile([P, ND, S], mm_dt, tag="hsT")       # [d%128, dtile, q]
            wT = {p: pers.tile([P, ND, GSZ], mm_dt, tag=f"wT{p}", name=f"wT{p}")
                  for p in "qkv"}
            maskT = pers.tile([P, NT], f32, tag="maskT")        # [k%128, ktile]
            biasT = pers.tile([P, 6], f32, tag="biasT")         # [dim%128, pair*2+proj(q,k)]
            qT = pers.tile([P, 3, S], mm_dt, tag="qT")          # [2*dh, pair, q]
            kT = pers.tile([P, 3, S], mm_dt, tag="kT")
            vsb = pers.tile([P, NT, HPC * DH1], mm_dt, tag="vsb")
            bvrow = pers.tile([1, GSZ], mm_dt, tag="bvrow")
            onesrow = pers.tile([1, P], mm_dt, tag="onesrow")

            nc.vector.memset(vsb[:], 1.0)     # ones columns; v dims overwritten
            nc.vector.memset(onesrow[:], 1.0)

            import contextlib
            loop_cm = (tc.For_i(0, loop_n, 1,
                                hint_engines=(mybir.EngineType.PE,
                                              mybir.EngineType.Activation,
                                              mybir.EngineType.DVE,
                                              mybir.EngineType.SP))
                       if loop_n else contextlib.nullcontext())
            with loop_cm:
                with tc.tile_pool(name="stage2", bufs=1) as stage2, \
                     tc.tile_pool(name="sa", bufs=2, space="PSUM") as sa_pool, \
                     tc.tile_pool(name="scr", bufs=1, space="PSUM") as scr_pool, \
                     tc.tile_pool(name="ctxp", bufs=3, space="PSUM") as ctx_pool, \
                     tc.tile_pool(name="prp", bufs=6) as pr_pool, \
                     tc.tile_pool(name="outsb", bufs=4) as out_pool:

                    # ---------- DMA issue (order = need order) ----------
                    mstage = stage2.tile([NT, P], f32, tag="mstage")
                    nc.sync.dma_start(mstage[:], mask_d[:])
                    bstage = stage2.tile([3, GSZ], f32, tag="bstage")
                    nc.sync.dma_start(bstage[:], bias_d[:])

                    # mask [NT, P] -> maskT [P, NT]
                    mps = scr_pool.tile([P, NT], f32, tag="scr", name="mps")
                    nc.tensor.transpose(mps[:], mstage[:], ident[:NT, :NT])
                    nc.vector.tensor_copy(maskT[:], mps[:])

                    # bias [3, GSZ] -> biasT [P, pair*2+proj]; bv -> bvrow
                    for pp in range(3):
                        bps = scr_pool.tile([P, 3], f32, tag="scr", name="bps")
                        nc.tensor.transpose(bps[:], bstage[:, pp * P:(pp + 1) * P],
                                            ident[:3, :3])
                        nc.vector.tensor_copy(biasT[:, pp * 2:pp * 2 + 2], bps[:, 0:2])
                    bvstage = stage2.tile([1, GSZ], f32, tag="bvstage")
                    nc.sync.dma_start(bvstage[:], bias_d[2:3, :])
                    nc.vector.tensor_copy(bvrow[0:1, :], bvstage[0:1, :])

                    # ---------- transpose / projection helpers ----------
                    # DMA-XBAR transposes straight from DRAM (16x128 tiles):
                    # no PE, DVE, or PSUM involvement at all.
                    def tp_hs(j):
                        nc.sync.dma_start_transpose(
                            hsT[:, :, j * P:(j + 1) * P],
                            hs_d[j * P:(j + 1) * P, :])

                    def tp_w(p, r):
                        nc.sync.dma_start_transpose(
                            wT[p][:, :, r * P:(r + 1) * P],
                            w_d[p][r * P:(r + 1) * P, :])

                    # kT/qT/V chunk projections: psum bank chosen per call.
                    # "scr" is the dedicated chain bank; "sa" borrows a score
                    # slot (free during the PE-bound qc0 window) so adjacent
                    # chains pipeline instead of serializing on one bank's
                    # WAR-vs-drain.
                    def _chain_ps(bank, name):
                        if bank == "sa":
                            t_ = sa_pool.tile([P, 2 * QC2], f32, tag="sa",
                                              name=name)
                            return t_[:, 0:QC2]
                        return scr_pool.tile([P, QC2], f32, tag="scr",
                                             name=name)[:]

                    def proj_qk(pname, dst, pp3, ch, pi, bank="scr"):
                        ps = _chain_ps(bank, f"pj{pname}{pp3}{ch}")
                        for d in range(ND):
                            nc.tensor.matmul(ps[:, 0:QC2],
                                             wT[pname][:, d, pp3 * P:(pp3 + 1) * P],
                                             hsT[:, d, ch * QC2:(ch + 1) * QC2],
                                             start=(d == 0), stop=(d == ND - 1))
                        nc.vector.tensor_scalar_add(
                            dst[:, pp3, ch * QC2:(ch + 1) * QC2],
                            ps[:, 0:QC2], biasT[:, pp3 * 2 + pi:pp3 * 2 + pi + 1])

                    def proj_v(t, bank="scr"):
                        ps = _chain_ps(bank, f"pv{t}")
                        for d in range(ND):
                            nc.tensor.matmul(ps[:, 0:GSZ],
                                             hsT[:, d, t * P:(t + 1) * P],
                                             wT["v"][:, d, :],
                                             start=(d == 0), stop=False)
                        nc.tensor.matmul(ps[:, 0:GSZ], onesrow[0:1, :],
                                         bvrow[0:1, :], start=False, stop=True)
                        nc.vector.tensor_copy(
                            vsb[:, t, :].rearrange("p (h c) -> p h c", c=DH1)[:, :, 0:DH],
                            ps[:, 0:GSZ].rearrange("p (h c) -> p h c", c=DH))

                    # ---------- A0: critical prep for the first scores ----------
                    # All DMA transposes issue upfront; the DMA queue works
                    # through them in this (need) order at ~0.7us each.
                    tp_w("q", 0); tp_w("k", 0)
                    for j in range(4):
                        tp_hs(j)
                    for r in range(3):
                        tp_w("v", r)
                    for j in range(4, 16):
                        tp_hs(j)
                    tp_w("q", 1); tp_w("k", 1)
                    tp_w("q", 2); tp_w("k", 2)
                    proj_qk("k", kT, 0, 0, 1, "scr")
                    proj_qk("q", qT, 0, 0, 0, "sa")
                    proj_v(0, "scr")

                    # ---------- deferred prep queue ----------
                    # Items carry an absolute "need-by" period (pair*64 +
                    # qc*16 + t): the item is force-issued by the END of
                    # period need-1 so the first consumer (period `need`)
                    # sees an earlier-issued producer. Uniform spreading
                    # drains ahead of the deadlines when PE has slack.
                    prep = []
                    def q_(need, fn, *a):
                        prep.append((need, fn, a))
                    # pair-0 qc0: kT chunks + V tiles. Chains alternate psum
                    # banks (scr/sa) so adjacent chains pipeline rather than
                    # serialize on one bank's drain.
                    q_(1, proj_v, 1, "sa"); q_(2, proj_v, 2, "scr")
                    q_(3, proj_v, 3, "sa"); q_(4, proj_qk, "k", kT, 0, 1, 1, "scr")
                    q_(4, proj_v, 4, "sa"); q_(5, proj_v, 5, "scr")
                    q_(6, proj_v, 6, "sa"); q_(7, proj_v, 7, "scr")
                    q_(8, proj_qk, "k", kT, 0, 2, 1, "sa"); q_(8, proj_v, 8, "scr")
                    q_(9, proj_v, 9, "sa"); q_(10, proj_v, 10, "scr")
                    q_(11, proj_v, 11, "sa")
                    q_(12, proj_qk, "k", kT, 0, 3, 1, "scr")
                    q_(12, proj_v, 12, "sa"); q_(13, proj_v, 13, "scr")
                    q_(14, proj_v, 14, "sa")
                    q_(15, proj_v, 15, "scr"); q_(16, proj_qk, "q", qT, 0, 1, 0, "sa")
                    n_qc0 = len(prep)
                    # pair-0 qc1: qT c2
                    q_(32, proj_qk, "q", qT, 0, 2, 0)
                    n_qc1 = len(prep)
                    # pair-0 qc2: qT c3 + pair-1 kT
                    q_(48, proj_qk, "q", qT, 0, 3, 0)
                    for c in range(4):
                        q_(64 + 4 * c, proj_qk, "k", kT, 1, c, 1)
                    n_qc2 = len(prep)
                    # pair-0 qc3: pair-1 qT
                    for c in range(4):
                        q_(64 + 16 * c if c else 64, proj_qk, "q", qT, 1, c, 0)
                    n_qc3 = len(prep)
                    n_qc4 = len(prep)
                    # pair-1 qc1: pair-2 kT
                    for c in range(4):
                        q_(128 + 4 * c, proj_qk, "k", kT, 2, c, 1)
                    n_qc5 = len(prep)
                    # pair-1 qc2: pair-2 qT
                    for c in range(4):
                        q_(128 + 16 * c if c else 128, proj_qk, "q", qT, 2, c, 0)
                    n_qc6 = len(prep)

                    # cumulative prep targets per (pair, qc)
                    targets = {(0, 0): n_qc0, (0, 1): n_qc1, (0, 2): n_qc2,
                               (0, 3): n_qc3, (1, 0): n_qc4, (1, 1): n_qc5,
                               (1, 2): n_qc6}
                    # suffix-min effective needs: an early item can never
                    # head-of-line-block a later item with a tighter deadline
                    # (issuing earlier is always safe; order is preserved)
                    eff = [0] * len(prep)
                    mn = 1 << 30
                    for idx in range(len(prep) - 1, -1, -1):
                        mn = min(mn, prep[idx][0])
                        eff[idx] = mn
                    prep = [(eff[idx], fn, a)
                            for idx, (need, fn, a) in enumerate(prep)]
                    state = {"done": 0}
                    mode = os.environ.get("BERT_SERIAL_PREP", "")
                    if mode:
                        # debug bisect: run selected prep kinds serially now,
                        # keep the rest interleaved
                        keep = []
                        for need, fn, a in prep:
                            kind = {tp_hs: "h", tp_w: "w", proj_v: "v",
                                    proj_qk: "k"}[fn]
                            if mode == "all" or kind in mode:
                                fn(*a)
                            else:
                                keep.append((need, fn, a))
                        prep[:] = keep

                    def drain_prep(p_abs, hi):
                        while state["done"] < len(prep):
                            need, fn, a = prep[state["done"]]
                            if state["done"] >= hi and need > p_abs + 1:
                                break
                            fn(*a)
                            state["done"] += 1

                    # ---------- Phase C: attention ----------
                    # 4 qtile accumulation regions share each ctx bank; psum
                    # start-bits zero a whole 2KB bank, so zero via DVE
                    # memset and accumulate with start=False instead. The
                    # memsets for window w+1 are issued mid-window-w so they
                    # never gate the next window's first PV.
                    def alloc_ctx(w):
                        tiles = {}
                        for i in (0, 1):
                            tiles[i] = ctx_pool.tile([P, QC2], f32, tag="ctx",
                                                     name=f"ctx{w}{i}")
                            nc.vector.memset(tiles[i][:], 0.0)
                        return tiles

                    def mk_pv(ctxs, pr, pp3, t, last):
                        def go():
                            for i in (0, 1):
                                h = 2 * pp3 + i
                                for j in range(QC2 // P):
                                    nc.tensor.matmul(
                                        ctxs[i][:, j * DH1:(j + 1) * DH1],
                                        pr[:, i * QC2 + j * P:
                                           i * QC2 + (j + 1) * P],
                                        vsb[:, t, h * DH1:(h + 1) * DH1],
                                        start=False, stop=last,
                                        skip_group_check=True)
                        return go

                    def mk_tail(ctxs, pp3, qc):
                        def go():
                            ot = out_pool.tile([P, QC2 // P, P], f32, tag="ot",
                                               name="ot")
                            for j in range(QC2 // P):
                                for i in (0, 1):
                                    rcp = out_pool.tile([P, 1], f32, tag="rcp",
                                                        name="rcp")
                                    nc.vector.reciprocal(
                                        rcp[:],
                                        ctxs[i][:, j * DH1 + DH:(j + 1) * DH1])
                                    nc.vector.tensor_scalar_mul(
                                        ot[:, j, i * DH:(i + 1) * DH],
                                        ctxs[i][:, j * DH1:j * DH1 + DH],
                                        rcp[:])
                            q0 = qc * QC2
                            nc.sync.dma_start(
                                out_d[q0:q0 + QC2, pp3 * P:(pp3 + 1) * P]
                                .rearrange("(j p) c -> p j c", p=P),
                                ot[:])
                        return go

                    # Window pipeline: the last NPEEL PV groups and the tail
                    # of window w are issued inside window w+1's first
                    # periods, so the exp(t15)->PV(t15)->scores(t0') chain
                    # never clogs the PE queue at a window boundary.
                    NPEEL = 2
                    pend = []
                    ctx_next = alloc_ctx(0)
                    for pp3 in range(3):
                        for qc in range(NQC):
                            w = pp3 * NQC + qc
                            base_done = state["done"]
                            tgt = targets.get((pp3, qc), state["done"])
                            ctxs = ctx_next
                            for t in range(NT):
                                sa = sa_pool.tile([P, 2 * QC2], f32, tag="sa",
                                                  name="sa")
                                for i in (0, 1):
                                    base = i * DH
                                    nc.tensor.matmul(
                                        sa[:, i * QC2:(i + 1) * QC2],
                                        kT[base:base + DH, pp3, t * P:(t + 1) * P],
                                        qT[base:base + DH, pp3,
                                           qc * QC2:(qc + 1) * QC2],
                                        start=True, stop=True)
                                pr = pr_pool.tile([P, 2 * QC2], mm_dt,
                                                  tag="pr", name="pr")
                                nc.scalar.activation(pr[:], sa[:], AF.Exp,
                                                     bias=maskT[:, t:t + 1],
                                                     scale=0.125)
                                if pend:
                                    pend.pop(0)()
                                if t >= NT - NPEEL:
                                    pend.append(mk_pv(ctxs, pr, pp3, t,
                                                      t == NT - 1))
                                else:
                                    mk_pv(ctxs, pr, pp3, t, False)()
                                # spread deferred prep across the t-loop
                                p_abs = pp3 * 64 + qc * NT + t
                                drain_prep(p_abs,
                                           base_done + ((tgt - base_done)
                                                        * (t + 1) + NT - 1) // NT)
                                if t == 12 and w + 1 < 3 * NQC:
                                    ctx_next = alloc_ctx(w + 1)
                            pend.append(mk_tail(ctxs, pp3, qc))
                    for fn_ in pend:
                        fn_()

    nc.compile()
    _cache[key] = nc
    return nc


def _in_maps(hidden_states, attention_mask, Wq, bq, Wk, bk, Wv, bv):
    import ml_dtypes
    mm_np = np.dtype(
        {"bfloat16": ml_dtypes.bfloat16, "float16": np.float16}.get(
            os.environ.get("BERT_MM_DT", "bfloat16"), ml_dtypes.bfloat16))
    maps = []
    for c in range(NCORES):
        b, g = c // 2, c % 2
        sl = slice(g * GSZ, (g + 1) * GSZ)
        maps.append({
            "hs": np.ascontiguousarray(
                np.asarray(hidden_states[b], dtype=np.float32).astype(mm_np)),
            "wq": np.ascontiguousarray(
                np.asarray(Wq[sl], dtype=np.float32).astype(mm_np)),
            "wk": np.ascontiguousarray(
                np.asarray(Wk[sl], dtype=np.float32).astype(mm_np)),
            "wv": np.ascontiguousarray(
                np.asarray(Wv[sl], dtype=np.float32).astype(mm_np)),
            "bias": np.ascontiguousarray(
                np.stack([bq[sl], bk[sl], bv[sl]]), dtype=np.float32),
            "mask": np.ascontiguousarray(
                attention_mask[b].reshape(NT, P), dtype=np.float32),
        })
    return maps


def kernel(hidden_states, attention_mask, Wq, bq, Wk, bk, Wv, bv,
           _trace=False, _tmpdir=None):
    from concourse.bass_utils import run_bass_kernel_spmd

    nc = _build(os.environ.get("BERT_MM_DT", "bfloat16"))
    maps = _in_maps(np.asarray(hidden_states), np.asarray(attention_mask),
                    np.asarray(Wq), np.asarray(bq), np.asarray(Wk),
                    np.asarray(bk), np.asarray(Wv), np.asarray(bv))
    res = run_bass_kernel_spmd(nc, maps, core_ids=list(range(NCORES)),
                               trace=_trace, tmpdir=_tmpdir)
    out = np.empty((B, S, D), dtype=np.float32)
    for c in range(NCORES):
        b, g = c // 2, c % 2
        out[b, :, g * GSZ:(g + 1) * GSZ] = res.results[c]["out"]
    kernel.last_results = res
    return out


# revision 38
# speedup vs baseline: 1.3116x; 1.0033x over previous
"""BertSelfAttention Trainium2 Bass kernel.

Full (unsharded) inputs in, full output out. Internally shards across 8
NeuronCores as (batch b, head-group g): core c handles batch c//2 and
heads [6*(c%2), 6*(c%2)+6) of the 12 heads.

Per-core program (Tile framework), designed to keep the Activation
engine (the exp bottleneck: 192 x [128,1024] exp instructions ~= 199us)
saturated:

  A) DMA hs/W/bias/mask; cast to bf16 on DVE, PE-transpose (1 cyc/row)
     into hsT [d,q] / wT [d,out]; mask/bias transposed in f32.
  B) kT/qT per head-pair via matmul chains (d-contraction in a shared
     score-psum slot), V [k, 65*(h)] chains (ones column appended per
     head for the softmax denominator, bias via rank-1 ones x bv).
  C) attention per pair, q-chunks of 512: per k-tile t, both heads'
     scoresT [k,512|512] land in one [128,1024] psum tile; one ACT
     instruction computes exp(0.125*s + mask_k) -> pr bf16 in SBUF.
     PV is output-stationary: ctx[q,65] accumulates pr^T (stationary)
     x [V_h | 1] over t — 65-row matmuls, no tail transpose. Tail:
     DVE reciprocal of the ones-column + scale, pair-batched DMA out.
  All phase-A/B prep that is not needed for the first scores is queued
  and drained inside the attention loop so the PE prep work hides
  behind ACT exp time instead of serializing in front of it.
"""

import os
import sys

sys.path.insert(0, "/opt/trn_rl_repo")

import numpy as np

B, S, D = 4, 2048, 768
H, DH = 12, 64
NCORES = 8
HPC = 6          # heads per core
GSZ = HPC * DH   # 384 output dims per core
P = 128
ND = D // P      # 6 d-tiles
NT = S // P      # 16 k-tiles
QC2 = 512        # q-chunk (scores matmul free dim; fp32-psum limit)
NQC = S // QC2   # 4
DH1 = DH + 1     # 65: v dims + ones column

_cache = {}


def _build(mm_dt_name: str, loop_n: int = 0):
    key = (mm_dt_name, loop_n)
    if key in _cache:
        return _cache[key]

    import concourse.bass as bass
    import concourse.bacc as bacc
    import concourse.mybir as mybir
    from concourse import tile
    from concourse.masks import make_identity

    f32 = mybir.dt.float32
    mm_dt = getattr(mybir.dt, mm_dt_name)
    AF = mybir.ActivationFunctionType

    nc = bacc.Bacc("TRN2", target_bir_lowering=False, debug=False,
                   num_devices=NCORES)

    # hs/W arrive pre-cast to the matmul dtype (host-side prep) so the DMA
    # XBAR can transpose straight out of DRAM (2-byte dtype requirement).
    hs_d = nc.dram_tensor("hs", [S, D], mm_dt, kind="ExternalInput")
    w_d = {p: nc.dram_tensor(f"w{p}", [GSZ, D], mm_dt, kind="ExternalInput")
           for p in "qkv"}
    bias_d = nc.dram_tensor("bias", [3, GSZ], f32, kind="ExternalInput")
    mask_d = nc.dram_tensor("mask", [NT, P], f32, kind="ExternalInput")
    out_d = nc.dram_tensor("out", [S, GSZ], f32, kind="ExternalOutput")
    dbg_d = (nc.dram_tensor("dbg", [QC2, 2 * DH1], f32, kind="ExternalOutput")
             if os.environ.get("BERT_DBG") else None)

    with tile.TileContext(nc) as tc:
        with tc.tile_pool(name="const", bufs=1) as const_pool, \
             tc.tile_pool(name="persist", bufs=1) as pers:

            ident = const_pool.tile([P, P], f32)
            make_identity(nc, ident[:])
            ident_mm = const_pool.tile([P, P], mm_dt)
            make_identity(nc, ident_mm[:])

            # ---- persistent SBUF tensors ----
            hsT = pers.tile([P, ND, S], mm_dt, tag="hsT")       # [d%128, dtile, q]
            wT = {p: pers.tile([P, ND, GSZ], mm_dt, tag=f"wT{p}", name=f"wT{p}")
                  for p in "qkv"}
            maskT = pers.tile([P, NT], f32, tag="maskT")        # [k%128, ktile]
            biasT = pers.tile([P, 6], f32, tag="biasT")         # [dim%128, pair*2+proj(q,k)]
            qT = pers.tile([P, 3, S], mm_dt, tag="qT")          # [2*dh, pair, q]
            kT = pers.tile([P, 3, S], mm_dt, tag="kT")
            vsb = pers.tile([P, NT, HPC * DH1], mm_dt, tag="vsb")
            bvrow = pers.tile([1, GSZ], mm_dt, tag="bvrow")
            onesrow = pers.tile([1, P], mm_dt, tag="onesrow")

            nc.vector.memset(vsb[:], 1.0)     # ones columns; v dims overwritten
            nc.vector.memset(onesrow[:], 1.0)

            import contextlib
            loop_cm = (tc.For_i(0, loop_n, 1,
                                hint_engines=(mybir.EngineType.PE,
                                              mybir.EngineType.Activation,
                                              mybir.EngineType.DVE,
                                              mybir.EngineType.SP))
                       if loop_n else contextlib.nullcontext())
            with loop_cm:
                with tc.tile_pool(name="stage2", bufs=1) as stage2, \
                     tc.tile_pool(name="sa", bufs=2, space="PSUM") as sa_pool, \
                     tc.tile_pool(name="scr", bufs=1, space="PSUM") as scr_pool, \
                     tc.tile_pool(name="ctxp", bufs=3, space="PSUM") as ctx_pool, \
                     tc.tile_pool(name="prp", bufs=6) as pr_pool, \
                     tc.tile_pool(name="outsb", bufs=4) as out_pool:

                    # ---------- DMA issue (order = need order) ----------
                    mstage = stage2.tile([NT, P], f32, tag="mstage")
                    nc.sync.dma_start(mstage[:], mask_d[:])
                    bstage = stage2.tile([3, GSZ], f32, tag="bstage")
                    nc.sync.dma_start(bstage[:], bias_d[:])

                    # mask [NT, P] -> maskT [P, NT]
                    mps = scr_pool.tile([P, NT], f32, tag="scr", name="mps")
                    nc.tensor.transpose(mps[:], mstage[:], ident[:NT, :NT])
                    nc.vector.tensor_copy(maskT[:], mps[:])

                    # bias [3, GSZ] -> biasT [P, pair*2+proj]; bv -> bvrow
                    for pp in range(3):
                        bps = scr_pool.tile([P, 3], f32, tag="scr", name="bps")
                        nc.tensor.transpose(bps[:], bstage[:, pp * P:(pp + 1) * P],
                                            ident[:3, :3])
                        nc.vector.tensor_copy(biasT[:, pp * 2:pp * 2 + 2], bps[:, 0:2])
                    bvstage = stage2.tile([1, GSZ], f32, tag="bvstage")
                    nc.sync.dma_start(bvstage[:], bias_d[2:3, :])
                    nc.vector.tensor_copy(bvrow[0:1, :], bvstage[0:1, :])

                    # ---------- transpose / projection helpers ----------
                    # DMA-XBAR transposes straight from DRAM (16x128 tiles):
                    # no PE, DVE, or PSUM involvement. Batched coarsely (4
                    # q-chunks for hs, one per W matrix) to amortize the
                    # per-instruction DMA overhead on hardware.
                    def tp_hs(c4):
                        nc.sync.dma_start_transpose(
                            hsT[:, :, c4 * 4 * P:(c4 + 1) * 4 * P],
                            hs_d[c4 * 4 * P:(c4 + 1) * 4 * P, :])

                    def tp_w(p):
                        nc.sync.dma_start_transpose(wT[p][:], w_d[p][:])

                    # kT/qT/V chunk projections: psum bank chosen per call.
                    # "scr" is the dedicated chain bank; "sa" borrows a score
                    # slot (free during the PE-bound qc0 window) so adjacent
                    # chains pipeline instead of serializing on one bank's
                    # WAR-vs-drain.
                    def _chain_ps(bank, name):
                        if bank == "sa":
                            t_ = sa_pool.tile([P, 2 * QC2], f32, tag="sa",
                                              name=name)
                            return t_[:, 0:QC2]
                        return scr_pool.tile([P, QC2], f32, tag="scr",
                                             name=name)[:]

                    def proj_qk(pname, dst, pp3, ch, pi, bank="scr"):
                        ps = _chain_ps(bank, f"pj{pname}{pp3}{ch}")
                        for d in range(ND):
                            nc.tensor.matmul(ps[:, 0:QC2],
                                             wT[pname][:, d, pp3 * P:(pp3 + 1) * P],
                                             hsT[:, d, ch * QC2:(ch + 1) * QC2],
                                             start=(d == 0), stop=(d == ND - 1))
                        nc.vector.tensor_scalar_add(
                            dst[:, pp3, ch * QC2:(ch + 1) * QC2],
                            ps[:, 0:QC2], biasT[:, pp3 * 2 + pi:pp3 * 2 + pi + 1])

                    def proj_v(t, bank="scr"):
                        ps = _chain_ps(bank, f"pv{t}")
                        for d in range(ND):
                            nc.tensor.matmul(ps[:, 0:GSZ],
                                             hsT[:, d, t * P:(t + 1) * P],
                                             wT["v"][:, d, :],
                                             start=(d == 0), stop=False)
                        nc.tensor.matmul(ps[:, 0:GSZ], onesrow[0:1, :],
                                         bvrow[0:1, :], start=False, stop=True)
                        nc.vector.tensor_copy(
                            vsb[:, t, :].rearrange("p (h c) -> p h c", c=DH1)[:, :, 0:DH],
                            ps[:, 0:GSZ].rearrange("p (h c) -> p h c", c=DH))

                    # ---------- A0: critical prep for the first scores ----------
                    # All DMA transposes issue upfront; the DMA queue works
                    # through them in this (need) order.
                    tp_w("q"); tp_w("k")
                    tp_hs(0)
                    tp_w("v")
                    for c4 in range(1, 4):
                        tp_hs(c4)
                    proj_qk("k", kT, 0, 0, 1, "scr")
                    proj_qk("q", qT, 0, 0, 0, "sa")
                    proj_v(0, "scr")

                    # ---------- deferred prep queue ----------
                    # Items carry an absolute "need-by" period (pair*64 +
                    # qc*16 + t): the item is force-issued by the END of
                    # period need-1 so the first consumer (period `need`)
                    # sees an earlier-issued producer. Uniform spreading
                    # drains ahead of the deadlines when PE has slack.
                    prep = []
                    def q_(need, fn, *a):
                        prep.append((need, fn, a))
                    # pair-0 qc0: kT chunks + V tiles. Chains alternate psum
                    # banks (scr/sa) so adjacent chains pipeline rather than
                    # serialize on one bank's drain.
                    q_(1, proj_v, 1, "sa"); q_(2, proj_v, 2, "scr")
                    q_(3, proj_v, 3, "sa"); q_(4, proj_qk, "k", kT, 0, 1, 1, "scr")
                    q_(4, proj_v, 4, "sa"); q_(5, proj_v, 5, "scr")
                    q_(6, proj_v, 6, "sa"); q_(7, proj_v, 7, "scr")
                    q_(8, proj_qk, "k", kT, 0, 2, 1, "sa"); q_(8, proj_v, 8, "scr")
                    q_(9, proj_v, 9, "sa"); q_(10, proj_v, 10, "scr")
                    q_(11, proj_v, 11, "sa")
                    q_(12, proj_qk, "k", kT, 0, 3, 1, "scr")
                    q_(12, proj_v, 12, "sa"); q_(13, proj_v, 13, "scr")
                    q_(14, proj_v, 14, "sa")
                    q_(15, proj_v, 15, "scr"); q_(16, proj_qk, "q", qT, 0, 1, 0, "sa")
                    n_qc0 = len(prep)
                    # pair-0 qc1: qT c2
                    q_(32, proj_qk, "q", qT, 0, 2, 0)
                    n_qc1 = len(prep)
                    # pair-0 qc2: qT c3 + pair-1 kT
                    q_(48, proj_qk, "q", qT, 0, 3, 0)
                    for c in range(4):
                        q_(64 + 4 * c, proj_qk, "k", kT, 1, c, 1)
                    n_qc2 = len(prep)
                    # pair-0 qc3: pair-1 qT
                    for c in range(4):
                        q_(64 + 16 * c if c else 64, proj_qk, "q", qT, 1, c, 0)
                    n_qc3 = len(prep)
                    n_qc4 = len(prep)  # (pair-2 W transposes already done upfront)
                    # pair-1 qc1: pair-2 kT
                    for c in range(4):
                        q_(128 + 4 * c, proj_qk, "k", kT, 2, c, 1)
                    n_qc5 = len(prep)
                    # pair-1 qc2: pair-2 qT
                    for c in range(4):
                        q_(128 + 16 * c if c else 128, proj_qk, "q", qT, 2, c, 0)
                    n_qc6 = len(prep)

                    # cumulative prep targets per (pair, qc)
                    targets = {(0, 0): n_qc0, (0, 1): n_qc1, (0, 2): n_qc2,
                               (0, 3): n_qc3, (1, 0): n_qc4, (1, 1): n_qc5,
                               (1, 2): n_qc6}
                    # suffix-min effective needs: an early item can never
                    # head-of-line-block a later item with a tighter deadline
                    # (issuing earlier is always safe; order is preserved)
                    eff = [0] * len(prep)
                    mn = 1 << 30
                    for idx in range(len(prep) - 1, -1, -1):
                        mn = min(mn, prep[idx][0])
                        eff[idx] = mn
                    prep = [(eff[idx], fn, a)
                            for idx, (need, fn, a) in enumerate(prep)]
                    state = {"done": 0}
                    mode = os.environ.get("BERT_SERIAL_PREP", "")
                    if mode:
                        # debug bisect: run selected prep kinds serially now,
                        # keep the rest interleaved
                        keep = []
                        for need, fn, a in prep:
                            kind = {tp_hs: "h", tp_w: "w", proj_v: "v",
                                    proj_qk: "k"}[fn]
                            if mode == "all" or kind in mode:
                                fn(*a)
                            else:
                                keep.append((need, fn, a))
                        prep[:] = keep

                    def drain_prep(p_abs, hi):
                        while state["done"] < len(prep):
                            need, fn, a = prep[state["done"]]
                            if state["done"] >= hi and need > p_abs + 1:
                                break
                            fn(*a)
                            state["done"] += 1

                    # ---------- Phase C: attention ----------
                    # 4 qtile accumulation regions share each ctx bank; psum
                    # start-bits zero a whole 2KB bank, so zero via DVE
                    # memset and accumulate with start=False instead. The
                    # memsets for window w+1 are issued mid-window-w so they
                    # never gate the next window's first PV.
                    def alloc_ctx(w):
                        tiles = {}
                        for i in (0, 1):
                            tiles[i] = ctx_pool.tile([P, QC2], f32, tag="ctx",
                                                     name=f"ctx{w}{i}")
                            nc.vector.memset(tiles[i][:], 0.0)
                        return tiles

                    def mk_pv(ctxs, pr, pp3, t, last):
                        def go():
                            for i in (0, 1):
                                h = 2 * pp3 + i
                                for j in range(QC2 // P):
                                    nc.tensor.matmul(
                                        ctxs[i][:, j * DH1:(j + 1) * DH1],
                                        pr[:, i * QC2 + j * P:
                                           i * QC2 + (j + 1) * P],
                                        vsb[:, t, h * DH1:(h + 1) * DH1],
                                        start=False, stop=last,
                                        skip_group_check=True)
                        return go

                    def mk_tail(ctxs, pp3, qc):
                        def go():
                            ot = out_pool.tile([P, QC2 // P, P], f32, tag="ot",
                                               name="ot")
                            for j in range(QC2 // P):
                                for i in (0, 1):
                                    rcp = out_pool.tile([P, 1], f32, tag="rcp",
                                                        name="rcp")
                                    nc.vector.reciprocal(
                                        rcp[:],
                                        ctxs[i][:, j * DH1 + DH:(j + 1) * DH1])
                                    nc.vector.tensor_scalar_mul(
                                        ot[:, j, i * DH:(i + 1) * DH],
                                        ctxs[i][:, j * DH1:j * DH1 + DH],
                                        rcp[:])
                            q0 = qc * QC2
                            nc.sync.dma_start(
                                out_d[q0:q0 + QC2, pp3 * P:(pp3 + 1) * P]
                                .rearrange("(j p) c -> p j c", p=P),
                                ot[:])
                        return go

                    # Window pipeline: the last NPEEL PV groups and the tail
                    # of window w are issued inside window w+1's first
                    # periods, so the exp(t15)->PV(t15)->scores(t0') chain
                    # never clogs the PE queue at a window boundary.
                    NPEEL = 2
                    pend = []
                    ctx_next = alloc_ctx(0)
                    for pp3 in range(3):
                        for qc in range(NQC):
                            w = pp3 * NQC + qc
                            base_done = state["done"]
                            tgt = targets.get((pp3, qc), state["done"])
                            ctxs = ctx_next
                            for t in range(NT):
                                sa = sa_pool.tile([P, 2 * QC2], f32, tag="sa",
                                                  name="sa")
                                for i in (0, 1):
                                    base = i * DH
                                    nc.tensor.matmul(
                                        sa[:, i * QC2:(i + 1) * QC2],
                                        kT[base:base + DH, pp3, t * P:(t + 1) * P],
                                        qT[base:base + DH, pp3,
                                           qc * QC2:(qc + 1) * QC2],
                                        start=True, stop=True)
                                pr = pr_pool.tile([P, 2 * QC2], mm_dt,
                                                  tag="pr", name="pr")
                                nc.scalar.activation(pr[:], sa[:], AF.Exp,
                                                     bias=maskT[:, t:t + 1],
                                                     scale=0.125)
                                if pend:
                                    pend.pop(0)()
                                if t >= NT - NPEEL:
                                    pend.append(mk_pv(ctxs, pr, pp3, t,
                                                      t == NT - 1))
                                else:
                                    mk_pv(ctxs, pr, pp3, t, False)()
                                # spread deferred prep across the t-loop
                                p_abs = pp3 * 64 + qc * NT + t
                                drain_prep(p_abs,
                                           base_done + ((tgt - base_done)
                                                        * (t + 1) + NT - 1) // NT)
                                if t == 12 and w + 1 < 3 * NQC:
                                    ctx_next = alloc_ctx(w + 1)
                            pend.append(mk_tail(ctxs, pp3, qc))
                    for fn_ in pend:
                        fn_()

    nc.compile()
    _cache[key] = nc
    return nc


def _in_maps(hidden_states, attention_mask, Wq, bq, Wk, bk, Wv, bv):
    import ml_dtypes
    mm_np = np.dtype(
        {"bfloat16": ml_dtypes.bfloat16, "float16": np.float16}.get(
            os.environ.get("BERT_MM_DT", "bfloat16"), ml_dtypes.bfloat16))
    maps = []
    for c in range(NCORES):
        b, g = c // 2, c % 2
        sl = slice(g * GSZ, (g + 1) * GSZ)
        maps.append({
            "hs": np.ascontiguousarray(
                np.asarray(hidden_states[b], dtype=np.float32).astype(mm_np)),
            "wq": np.ascontiguousarray(
                np.asarray(Wq[sl], dtype=np.float32).astype(mm_np)),
            "wk": np.ascontiguousarray(
                np.asarray(Wk[sl], dtype=np.float32).astype(mm_np)),
            "wv": np.ascontiguousarray(
                np.asarray(Wv[sl], dtype=np.float32).astype(mm_np)),
            "bias": np.ascontiguousarray(
                np.stack([bq[sl], bk[sl], bv[sl]]), dtype=np.float32),
            "mask": np.ascontiguousarray(
                attention_mask[b].reshape(NT, P), dtype=np.float32),
        })
    return maps


def kernel(hidden_states, attention_mask, Wq, bq, Wk, bk, Wv, bv,
           _trace=False, _tmpdir=None):
    from concourse.bass_utils import run_bass_kernel_spmd

    nc = _build(os.environ.get("BERT_MM_DT", "bfloat16"))
    maps = _in_maps(np.asarray(hidden_states), np.asarray(attention_mask),
                    np.asarray(Wq), np.asarray(bq), np.asarray(Wk),
                    np.asarray(bk), np.asarray(Wv), np.asarray(bv))
    res = run_bass_kernel_spmd(nc, maps, core_ids=list(range(NCORES)),
                               trace=_trace, tmpdir=_tmpdir)
    out = np.empty((B, S, D), dtype=np.float32)
    for c in range(NCORES):
        b, g = c // 2, c % 2
        out[b, :, g * GSZ:(g + 1) * GSZ] = res.results[c]["out"]
    kernel.last_results = res
    return out
